# revision 14
# baseline (speedup 1.0000x reference)
"""GQA attention kernel (B=1, S=2048, D=4096, 32 Q heads / 8 KV heads, RoPE,
causal) for 8 Trainium2 NeuronCores.

Sharding: tensor-parallel over heads. Core c owns Q heads 4c..4c+3 and KV head
c (whole GQA group), computes its context slice and a partial o-projection
(rows 512c..512c+511 of Wo); the host sums the 8 partial outputs.

v2 changes vs baseline:
- all DRAM tensors pre-tiled on host so every DMA is per-partition contiguous
- QKV weights shipped bf16, upconverted to fp32r on DVE (halves weight DMA)
- rowsum via DVE accumulation of exp tiles + one ones-column matmul per
  (head, chunk) instead of a 512-cycle matmul per key tile
- rotate_half via DVE partition-shifted copies + sign-folded sin table
  (no PE matmul)
- causal diagonal tiles width-restricted (N>=256 kept for fp32r full rate)
- engine split: PE matmul only / scalar exp only / DVE elementwise
- o-projection from SBUF-resident bf16 context, bf16 Wo, bf16 output
  (host upconverts + sums); no ctx DRAM roundtrip
"""
import numpy as np
import ml_dtypes
from contextlib import ExitStack

try:  # reuse compiled executables across processes when possible
    import jax
    jax.config.update("jax_compilation_cache_dir", "/tmp/jax_comp_cache")
    jax.config.update("jax_persistent_cache_min_entry_size_bytes", -1)
    jax.config.update("jax_persistent_cache_min_compile_time_secs", 1.0)
except Exception:
    pass

import concourse.bacc as bacc
import concourse.tile as tile
import concourse.mybir as mybir
from concourse.bass_utils import run_bass_kernel_spmd

F32 = mybir.dt.float32
F32R = mybir.dt.float32r
BF16 = mybir.dt.bfloat16

S = 2048            # sequence length
D = 4096            # hidden dim
HD = 128            # head dim
NCORES = 8
QH = 4              # q heads per core
KT = D // 128       # 32 contraction tiles for the projections
NCHUNK = S // 512   # 4 sequence chunks of 512
NJT = S // 128      # 16 seq tiles of 128
INV_SQRT_D = float(1.0 / np.sqrt(np.float32(HD)))
NEG_INF = -3.4e38
ROPE_BASE = 10000.0


def round_fp32r(x: np.ndarray) -> np.ndarray:
    """Round fp32 to fp32r (11 mantissa bits, RNE); low 12 bits zeroed."""
    u = np.ascontiguousarray(x, dtype=np.float32).view(np.uint32)
    keep = (u >> 12) & np.uint32(1)
    u = u + np.uint32(0x7FF) + keep
    u = u & np.uint32(0xFFFFF000)
    return u.view(np.float32)


def _build_nc():
    nc = bacc.Bacc(None)

    # pre-tiled bf16 inputs: hst rows ((icnk*16 + tp)*128 + p) hold the
    # t-PAIR (2*tp, 2*tp+1) side by side -> one DMA per two tiles
    hst_d = nc.dram_tensor("hst", [NCHUNK * 16 * 128, 1024], BF16, kind="ExternalInput")
    # weights bf16, partition-major [128, KT, m] so group loads are contiguous
    wqb_d = nc.dram_tensor("wqb", [128, KT, QH * HD], BF16, kind="ExternalInput")
    wkb_d = nc.dram_tensor("wkb", [128, KT, HD], BF16, kind="ExternalInput")
    wvb_d = nc.dram_tensor("wvb", [128, KT, HD], BF16, kind="ExternalInput")
    wob_d = nc.dram_tensor("wob", [4 * 128, D], BF16, kind="ExternalInput")
    # cos / sign-folded sin, partition-major per chunk [128, NCHUNK, 512]
    cos_d = nc.dram_tensor("cost", [128, NCHUNK, 512], F32, kind="ExternalInput")
    sinn_d = nc.dram_tensor("sinn", [128, NCHUNK, 512], F32, kind="ExternalInput")
    mask_d = nc.dram_tensor("maskt", [128, 4, 512], BF16, kind="ExternalInput")
    ident_d = nc.dram_tensor("ident", [128, 128], F32R, kind="ExternalInput")
    ones_d = nc.dram_tensor("ones", [128, 128], F32R, kind="ExternalInput")
    # output bf16, tiled rows ((st*8 + ec)*128 + p)
    out_d = nc.dram_tensor("out", [NJT * 8 * 128, 512], BF16, kind="ExternalOutput")

    with tile.TileContext(nc) as tc, ExitStack() as ctx:
        wpool = ctx.enter_context(tc.tile_pool(name="wpool", bufs=1))
        cpool = ctx.enter_context(tc.tile_pool(name="cpool", bufs=1))
        big = ctx.enter_context(tc.tile_pool(name="bigacts", bufs=1))
        trig = ctx.enter_context(tc.tile_pool(name="trig", bufs=2))
        hsp = ctx.enter_context(tc.tile_pool(name="hsp", bufs=4))
        chp = ctx.enter_context(tc.tile_pool(name="chp", bufs=4))
        rotp = ctx.enter_context(tc.tile_pool(name="rotp", bufs=1))
        qrp = ctx.enter_context(tc.tile_pool(name="qrp", bufs=4))
        tmpp = ctx.enter_context(tc.tile_pool(name="tmpp", bufs=1))
        ptp = ctx.enter_context(tc.tile_pool(name="ptp", bufs=5))
        rsp = ctx.enter_context(tc.tile_pool(name="rsp", bufs=2))
        smal = ctx.enter_context(tc.tile_pool(name="smal", bufs=1))
        rbp = ctx.enter_context(tc.tile_pool(name="rbp", bufs=2))
        osb = ctx.enter_context(tc.tile_pool(name="osb", bufs=4))
        psum = ctx.enter_context(tc.tile_pool(name="psum", bufs=8, space="PSUM"))

        # ---- resident weights & constants (bf16, loaded directly) ----
        wq_sb = wpool.tile([128, KT, QH * HD], BF16, tag="wq")
        wk_sb = wpool.tile([128, KT, HD], BF16, tag="wk")
        wv_sb = wpool.tile([128, KT, HD], BF16, tag="wv")
        wo_sb = wpool.tile([128, 4, D], BF16, tag="wo")

        mask_sb = cpool.tile([128, 4, 512], BF16, tag="mask")
        ident_sb = cpool.tile([128, 128], F32R, tag="ident")
        ones_sb = cpool.tile([128, 128], F32R, tag="ones")

        krope_sb = big.tile([128, S], F32R, tag="krope")   # kT after rope
        vnat_sb = big.tile([128, S], F32R, tag="vnat")     # v natural [j, d] blocks
        ctx_sb = big.tile([128, QH, S], BF16, tag="ctx")   # normalized context^T

        # --- weight group loads (scalar queue; chunk-0 only) ---
        def wq_load(g):  # 4 groups of 8 t-tiles, 1 MB each
            nc.scalar.dma_start(out=wq_sb[:, 8 * g:8 * g + 8, :],
                                in_=wqb_d[:, 8 * g:8 * g + 8, :])

        def wkv_load(g):  # 2 groups of 16 t-tiles each
            nc.scalar.dma_start(out=wk_sb[:, 16 * g:16 * g + 16, :],
                                in_=wkb_d[:, 16 * g:16 * g + 16, :])
            nc.scalar.dma_start(out=wv_sb[:, 16 * g:16 * g + 16, :],
                                in_=wvb_d[:, 16 * g:16 * g + 16, :])

        wq_load(0)
        wkv_load(0)
        wq_load(1)

        def _late_loads(t):
            # chunk-0 only: stream remaining weights + tables
            if t == 2:
                wkv_load(1)
            elif t == 4:
                wq_load(2)
            elif t == 8:
                wq_load(3)
            elif t == 12:
                nc.gpsimd.dma_start(out=mask_sb[:], in_=mask_d[:, :, :])
            elif t == 16:
                nc.gpsimd.dma_start(out=ident_sb[:], in_=ident_d[:, :])
                nc.gpsimd.dma_start(out=ones_sb[:], in_=ones_d[:, :])

        hst_prefetched = {}

        def hst_pair_load(icnk, tp):
            key = (icnk, tp)
            if key in hst_prefetched:
                return hst_prefetched.pop(key)
            h = hsp.tile([128, 2, 512], BF16, tag="hst", name=f"hst{icnk}_{tp}")
            r0 = (icnk * 16 + tp) * 128
            nc.sync.dma_start(out=h[:], in_=hst_d[r0:r0 + 128, :])
            return h

        # ---- fused per-chunk pipeline ----
        for icnk in range(NCHUNK):
            c0, c1 = icnk * 512, (icnk + 1) * 512

            # per-chunk trig slices (gpsimd queue, contiguous per partition)
            cos_t = trig.tile([128, 512], F32, tag="cos", name=f"cos{icnk}")
            sinn_t = trig.tile([128, 512], F32, tag="sinn", name=f"sinn{icnk}")
            nc.gpsimd.dma_start(out=cos_t[:], in_=cos_d[:, icnk, :])
            nc.gpsimd.dma_start(out=sinn_t[:], in_=sinn_d[:, icnk, :])

            # joint qkv projection for this chunk: 6 accumulators (q0..q3, k, v)
            accs = [psum.tile([128, 512], F32, tag="ps", name=f"acc{icnk}_{i}")
                    for i in range(6)]
            for tp in range(16):
                hst_t = hst_pair_load(icnk, tp)
                if icnk == 0:
                    _late_loads(2 * tp)
                for tt in range(2):
                    t = 2 * tp + tt
                    # last iteration: finish k, then q0, then v, so their
                    # evac/rope chains start while the PE drains the rest
                    morder = ((4, 0, 5, 1, 2, 3) if t == KT - 1
                              else (0, 1, 2, 3, 4, 5))
                    for m in morder:
                        if m < 4:
                            lhsT = wq_sb[:, t, m * HD:(m + 1) * HD]
                        elif m == 4:
                            lhsT = wk_sb[:, t, :]
                        else:
                            lhsT = wv_sb[:, t, :]
                        nc.tensor.matmul(accs[m][:], lhsT, hst_t[:, tt, :],
                                         start=(t == 0), stop=(t == KT - 1))

            def evac(m, eng="v"):
                ch = chp.tile([128, 512], F32R, tag="ch", name=f"ch{icnk}_{m}")
                if eng == "s":  # scalar engine: parallel to DVE at boundaries
                    nc.scalar.copy(ch[:], accs[m][:])
                else:
                    nc.vector.tensor_copy(ch[:], accs[m][:])
                return ch

            def rope_into(ch, dest_ap, name):
                # rotate_half via partition-shifted copies; sign folded in sinn
                rot = rotp.tile([128, 512], F32, tag="rot", name=f"rot{name}")
                nc.vector.tensor_copy(rot[0:64, :], ch[64:128, :].bitcast(F32))
                nc.vector.tensor_copy(rot[64:128, :], ch[0:64, :].bitcast(F32))
                t1 = tmpp.tile([128, 512], F32, tag="t1", name=f"t1{name}")
                nc.vector.tensor_mul(t1[:], ch[:].bitcast(F32), cos_t[:])
                t2 = tmpp.tile([128, 512], F32, tag="t2", name=f"t2{name}")
                nc.vector.tensor_mul(t2[:], rot[:], sinn_t[:])
                nc.vector.tensor_add(dest_ap, t1[:], t2[:])

            def rope_q(m):
                qr = qrp.tile([128, 512], F32R, tag="qrp", name=f"qr{icnk}_{m}")
                rope_into(chs[m], qr[:], f"q{icnk}_{m}")
                return qr

            # evacuate ALL psum accumulators upfront (frees banks for the
            # attention tiles; lazy evac deadlocks the 8-slot psum rotation),
            # but compose ropes lazily per head so DVE runs just ahead of PE.
            # evacs for k/q0/v on the scalar engine (idle at boundaries) so
            # the DVE can start rope immediately; q1-3 evac on DVE after
            qrs = [None] * QH
            chs = {}
            if icnk == 0:
                chs[4] = evac(4, "s")
                chs[0] = evac(0, "s")
                chs[5] = evac(5, "s")
                rope_into(chs[4], krope_sb[:, c0:c1], f"k{icnk}")
                qrs[0] = qrp.tile([128, 512], F32R, tag="qrp",
                                  name=f"qr{icnk}_0")
                rope_into(chs[0], qrs[0][:], f"q{icnk}_0")
            else:
                chs[0] = evac(0, "s")
                chs[4] = evac(4, "s")
                chs[5] = evac(5, "s")
                qrs[0] = qrp.tile([128, 512], F32R, tag="qrp",
                                  name=f"qr{icnk}_0")
                rope_into(chs[0], qrs[0][:], f"q{icnk}_0")
                rope_into(chs[4], krope_sb[:, c0:c1], f"k{icnk}")
            for m in (1, 2, 3):
                chs[m] = evac(m)

            # v transpose: 4 sub-blocks into one psum bank, then DVE to vnat
            ch_v = chs[5]
            vt_ps = psum.tile([128, 512], F32R, tag="ps", name=f"vt{icnk}")
            for tt in range(4):
                nc.tensor.matmul(vt_ps[:, tt * 128:(tt + 1) * 128],
                                 ch_v[:, tt * 128:(tt + 1) * 128],
                                 ident_sb[:], is_transpose=True,
                                 start=True, stop=True)
            for tt in range(4):
                jt = icnk * 4 + tt
                nc.vector.tensor_copy(vnat_sb[:, jt * 128:(jt + 1) * 128],
                                      vt_ps[:, tt * 128:(tt + 1) * 128])

            # attention for the 4 heads, query chunk = icnk (keys 0..4icnk+3)
            def attention(h):
                qr = qrs[h]
                ctx_acc = psum.tile([128, 512], F32, tag="ps",
                                    name=f"ctx{icnk}_{h}")
                acc_rs = rsp.tile([128, 512], F32R, tag="rs",
                                  name=f"rs{icnk}_{h}")
                jt_max = icnk * 4 + 3
                pending = []
                LOOKAHEAD = 2

                def consume(item, last):
                    jt, lo, pT = item
                    nc.tensor.matmul(ctx_acc[:, lo:512],
                                     vnat_sb[:, jt * 128:(jt + 1) * 128],
                                     pT[:, lo:512],
                                     start=(jt == 0), stop=last,
                                     skip_group_check=True)

                for jt in range(jt_max + 1):
                    r = jt - icnk * 4
                    lo = 0 if r <= 0 else (128 if r == 1 else 256)
                    sT = psum.tile([128, 512], F32, tag="ps",
                                   name=f"sT{icnk}_{h}_{jt}")
                    nc.tensor.matmul(sT[:, lo:512],
                                     krope_sb[:, jt * 128:(jt + 1) * 128],
                                     qr[:, lo:512], start=True, stop=True)
                    if len(pending) >= LOOKAHEAD:
                        consume(pending.pop(0), False)
                    if r >= 0:  # diagonal: apply causal mask
                        nc.vector.tensor_add(sT[:, lo:512], sT[:, lo:512],
                                             mask_sb[:, r, lo:512])
                    pT = ptp.tile([128, 512], F32R, tag="pt",
                                  name=f"pt{icnk}_{h}_{jt}")
                    nc.scalar.activation(out=pT[:, lo:512], in_=sT[:, lo:512],
                                         func=mybir.ActivationFunctionType.Exp,
                                         scale=INV_SQRT_D)
                    # rowsum accumulate on DVE
                    if jt == 0:
                        nc.vector.tensor_copy(acc_rs[:, :], pT[:, :])
                    else:
                        nc.vector.tensor_add(acc_rs[:, lo:512],
                                             acc_rs[:, lo:512], pT[:, lo:512])
                    pending.append((jt, lo, pT))
                while pending:
                    consume(pending.pop(0), len(pending) == 0)

                # rowsum + normalize; broadcast on gpsimd, off the PE path
                rs_ps = psum.tile([1, 512], F32, tag="ps",
                                  name=f"rsps{icnk}_{h}")
                nc.tensor.matmul(rs_ps[:], ones_sb[:, 0:1], acc_rs[:],
                                 start=True, stop=True)
                recip = smal.tile([1, 512], F32, tag="recip")
                with nc.allow_low_precision(reason="softmax denom reciprocal"):
                    nc.vector.reciprocal(recip[:], rs_ps[:])
                rb = rbp.tile([128, 512], F32, tag="rb", name=f"rb{icnk}_{h}")
                nc.gpsimd.partition_broadcast(rb[:], recip[:])
                nc.vector.tensor_mul(ctx_sb[:, h, c0:c1], ctx_acc[:], rb[:])

            for h in range(QH):
                if h + 1 < QH:
                    qrs[h + 1] = rope_q(h + 1)  # DVE runs ahead of the PE here
                attention(h)
                if icnk < NCHUNK - 1 and h == 1:
                    # prefetch next chunk's first hst pairs during attention
                    for tpre in range(3):
                        hh = hsp.tile([128, 2, 512], BF16, tag="hst",
                                      name=f"hst{icnk + 1}_{tpre}")
                        r0 = ((icnk + 1) * 16 + tpre) * 128
                        nc.sync.dma_start(out=hh[:], in_=hst_d[r0:r0 + 128, :])
                        hst_prefetched[(icnk + 1, tpre)] = hh
                if icnk == NCHUNK - 2 and h == 2:
                    # load bf16 Wo (own slot) well before the o-projection
                    for jt in range(4):
                        nc.gpsimd.dma_start(
                            out=wo_sb[:, jt, :],
                            in_=wob_d[jt * 128:(jt + 1) * 128, :])

        # ---- partial o-projection from SBUF ctx (bf16), bf16 out ----
        # out written in ec-PAIRS on alternating queues: the per-DMA issue
        # cost (~1.4us) would otherwise gate the o-proj phase
        out_r = out_d[:, :].rearrange("(n p) f -> p n f", p=128)
        for st in range(NJT):
            for ecp in range(4):  # pairs of ec
                ot = osb.tile([128, 2, 512], BF16, tag="ot",
                              name=f"ot{st}_{ecp}")
                for half in range(2):
                    ec = ecp * 2 + half
                    oacc = psum.tile([128, 512], F32, tag="ps",
                                     name=f"o{st}_{ec}")
                    for jt in range(4):
                        nc.tensor.matmul(
                            oacc[:],
                            ctx_sb[:, jt, st * 128:(st + 1) * 128],
                            wo_sb[:, jt, ec * 512:(ec + 1) * 512],
                            start=(jt == 0), stop=(jt == 3))
                    nc.scalar.copy(ot[:, half, :], oacc[:])
                n0 = st * 8 + ecp * 2
                eng = nc.sync if ecp % 2 == 0 else nc.gpsimd
                eng.dma_start(out=out_r[:, n0:n0 + 2, :], in_=ot[:])

    nc.finalize()
    return nc


_NC_CACHE = None


def _host_tables():
    inv_freq = 1.0 / (ROPE_BASE ** (np.arange(0, HD, 2, dtype=np.float32) / HD))
    pos = np.arange(S, dtype=np.float32)
    freqs = pos[:, None] * inv_freq[None, :].astype(np.float32)   # [S, 64]
    emb = np.concatenate([freqs, freqs], axis=1).astype(np.float32)  # [S, 128]
    cosT = np.ascontiguousarray(np.cos(emb).astype(np.float32).T)  # [128, S]
    sinT = np.ascontiguousarray(np.sin(emb).astype(np.float32).T)
    sinN = sinT.copy()
    sinN[0:64, :] *= -1.0  # sign folded: rot_abs[0:64]=x2, [64:]=x1

    # per-chunk partition-major trig: [128, NCHUNK, 512]
    cosPM = np.ascontiguousarray(cosT.reshape(128, NCHUNK, 512))
    sinnPM = np.ascontiguousarray(sinN.reshape(128, NCHUNK, 512))

    # causal mask for diagonal blocks, transposed orientation [jp, r, if]
    jp = np.arange(128)[:, None, None]
    r = np.arange(4)[None, :, None]
    iF = np.arange(512)[None, None, :]
    mask = np.where(r * 128 + jp <= iF, 0.0, NEG_INF).astype(np.float32)
    mask_bf = mask.astype(ml_dtypes.bfloat16)

    ident = np.eye(128, dtype=np.float32)
    ones = np.ones((128, 128), dtype=np.float32)
    return cosPM, sinnPM, mask_bf, ident, ones


def kernel(hidden_states, Wq, Wk, Wv, Wo):
    global _NC_CACHE
    if _NC_CACHE is None:
        _NC_CACHE = _build_nc()
    nc = _NC_CACHE

    hs = np.asarray(hidden_states, dtype=np.float32)
    B = hs.shape[0]
    assert hs.shape == (B, S, D)
    hst = np.ascontiguousarray(hs[0].T).astype(ml_dtypes.bfloat16)  # [D, S]
    # pair-tiled: row (icnk*16 + tp)*128 + p = [t=2tp | t=2tp+1] halves
    hst_t = np.ascontiguousarray(
        hst.reshape(16, 2, 128, NCHUNK, 512).transpose(3, 0, 2, 1, 4)
    ).reshape(NCHUNK * 16 * 128, 1024)
    cosPM, sinnPM, mask_bf, ident, ones = _host_tables()

    Wq = np.asarray(Wq, dtype=np.float32)
    Wk = np.asarray(Wk, dtype=np.float32)
    Wv = np.asarray(Wv, dtype=np.float32)
    Wo = np.asarray(Wo, dtype=np.float32)

    def pm_bf16(w):  # [D, m] -> partition-major [128, KT, m] bf16
        m = w.shape[1]
        return np.ascontiguousarray(
            w.reshape(KT, 128, m).transpose(1, 0, 2)).astype(ml_dtypes.bfloat16)

    in_maps = []
    for c in range(NCORES):
        in_maps.append({
            "hst": hst_t,
            "wqb": pm_bf16(Wq[:, c * QH * HD:(c + 1) * QH * HD]),
            "wkb": pm_bf16(Wk[:, c * HD:(c + 1) * HD]),
            "wvb": pm_bf16(Wv[:, c * HD:(c + 1) * HD]),
            "wob": np.ascontiguousarray(
                Wo[c * QH * HD:(c + 1) * QH * HD, :]).astype(ml_dtypes.bfloat16),
            "cost": cosPM,
            "sinn": sinnPM,
            "maskt": mask_bf,
            "ident": ident,
            "ones": ones,
        })

    import os
    trace = os.environ.get("KERNEL_TRACE") == "1"
    if trace:
        try:
            import antenv.axon_hooks  # noqa: F401  (profiling hook, optional)
        except ImportError:
            trace = False
    res = run_bass_kernel_spmd(nc, in_maps, list(range(NCORES)), trace=trace)
    if trace:
        kernel.last_results = res

    acc = np.zeros((NJT, 8, 128, 512), dtype=np.float64)
    for c in range(NCORES):
        acc += res.results[c]["out"].astype(np.float64).reshape(NJT, 8, 128, 512)
    # [st, ec, p, f] -> [st, p, ec, f] -> [S, D]
    out = acc.transpose(0, 2, 1, 3).reshape(S, D).astype(np.float32)
    return out.reshape(B, S, D)


# revision 15
# speedup vs baseline: 1.0498x; 1.0498x over previous
"""GQA attention kernel (B=1, S=2048, D=4096, 32 Q heads / 8 KV heads, RoPE,
causal) for 8 Trainium2 NeuronCores.

Sharding: tensor-parallel over heads. Core c owns Q heads 4c..4c+3 and KV head
c (whole GQA group), computes its context slice and a partial o-projection
(rows 512c..512c+511 of Wo); the host sums the 8 partial outputs.

v2 changes vs baseline:
- all DRAM tensors pre-tiled on host so every DMA is per-partition contiguous
- QKV weights shipped bf16, upconverted to fp32r on DVE (halves weight DMA)
- rowsum via DVE accumulation of exp tiles + one ones-column matmul per
  (head, chunk) instead of a 512-cycle matmul per key tile
- rotate_half via DVE partition-shifted copies + sign-folded sin table
  (no PE matmul)
- causal diagonal tiles width-restricted (N>=256 kept for fp32r full rate)
- engine split: PE matmul only / scalar exp only / DVE elementwise
- o-projection from SBUF-resident bf16 context, bf16 Wo, bf16 output
  (host upconverts + sums); no ctx DRAM roundtrip
"""
import numpy as np
import ml_dtypes
from contextlib import ExitStack

try:  # reuse compiled executables across processes when possible
    import jax
    jax.config.update("jax_compilation_cache_dir", "/tmp/jax_comp_cache")
    jax.config.update("jax_persistent_cache_min_entry_size_bytes", -1)
    jax.config.update("jax_persistent_cache_min_compile_time_secs", 1.0)
except Exception:
    pass

import concourse.bacc as bacc
import concourse.tile as tile
import concourse.mybir as mybir
from concourse.bass_utils import run_bass_kernel_spmd

F32 = mybir.dt.float32
F32R = mybir.dt.float32r
BF16 = mybir.dt.bfloat16

S = 2048            # sequence length
D = 4096            # hidden dim
HD = 128            # head dim
NCORES = 8
QH = 4              # q heads per core
KT = D // 128       # 32 contraction tiles for the projections
NCHUNK = S // 512   # 4 sequence chunks of 512
NJT = S // 128      # 16 seq tiles of 128
INV_SQRT_D = float(1.0 / np.sqrt(np.float32(HD)))
NEG_INF = -3.4e38
ROPE_BASE = 10000.0


def round_fp32r(x: np.ndarray) -> np.ndarray:
    """Round fp32 to fp32r (11 mantissa bits, RNE); low 12 bits zeroed."""
    u = np.ascontiguousarray(x, dtype=np.float32).view(np.uint32)
    keep = (u >> 12) & np.uint32(1)
    u = u + np.uint32(0x7FF) + keep
    u = u & np.uint32(0xFFFFF000)
    return u.view(np.float32)


def _build_nc():
    nc = bacc.Bacc(None)

    # pre-tiled bf16 inputs: hst rows ((icnk*16 + tp)*128 + p) hold the
    # t-PAIR (2*tp, 2*tp+1) side by side -> one DMA per two tiles
    hst_d = nc.dram_tensor("hst", [NCHUNK * 16 * 128, 1024], BF16, kind="ExternalInput")
    # weights bf16, partition-major [128, KT, m] so group loads are contiguous
    wqb_d = nc.dram_tensor("wqb", [128, KT, QH * HD], BF16, kind="ExternalInput")
    wkb_d = nc.dram_tensor("wkb", [128, KT, HD], BF16, kind="ExternalInput")
    wvb_d = nc.dram_tensor("wvb", [128, KT, HD], BF16, kind="ExternalInput")
    wob_d = nc.dram_tensor("wob", [4 * 128, D], BF16, kind="ExternalInput")
    # cos / sign-folded sin, partition-major per chunk [128, NCHUNK, 512]
    cos_d = nc.dram_tensor("cost", [128, NCHUNK, 512], F32, kind="ExternalInput")
    sinn_d = nc.dram_tensor("sinn", [128, NCHUNK, 512], F32, kind="ExternalInput")
    mask_d = nc.dram_tensor("maskt", [128, 4, 512], BF16, kind="ExternalInput")
    ident_d = nc.dram_tensor("ident", [128, 128], F32R, kind="ExternalInput")
    ones_d = nc.dram_tensor("ones", [128, 128], F32R, kind="ExternalInput")
    # output bf16, tiled rows ((st*8 + ec)*128 + p)
    out_d = nc.dram_tensor("out", [NJT * 8 * 128, 512], BF16, kind="ExternalOutput")

    with tile.TileContext(nc) as tc, ExitStack() as ctx:
        wpool = ctx.enter_context(tc.tile_pool(name="wpool", bufs=1))
        cpool = ctx.enter_context(tc.tile_pool(name="cpool", bufs=1))
        big = ctx.enter_context(tc.tile_pool(name="bigacts", bufs=1))
        trig = ctx.enter_context(tc.tile_pool(name="trig", bufs=2))
        hsp = ctx.enter_context(tc.tile_pool(name="hsp", bufs=4))
        chp = ctx.enter_context(tc.tile_pool(name="chp", bufs=4))
        rotp = ctx.enter_context(tc.tile_pool(name="rotp", bufs=1))
        qrp = ctx.enter_context(tc.tile_pool(name="qrp", bufs=4))
        tmpp = ctx.enter_context(tc.tile_pool(name="tmpp", bufs=1))
        ptp = ctx.enter_context(tc.tile_pool(name="ptp", bufs=5))
        rsp = ctx.enter_context(tc.tile_pool(name="rsp", bufs=2))
        smal = ctx.enter_context(tc.tile_pool(name="smal", bufs=1))
        rbp = ctx.enter_context(tc.tile_pool(name="rbp", bufs=2))
        osb = ctx.enter_context(tc.tile_pool(name="osb", bufs=4))
        psum = ctx.enter_context(tc.tile_pool(name="psum", bufs=8, space="PSUM"))

        # ---- resident weights & constants (bf16, loaded directly) ----
        wq_sb = wpool.tile([128, KT, QH * HD], BF16, tag="wq")
        wk_sb = wpool.tile([128, KT, HD], BF16, tag="wk")
        wv_sb = wpool.tile([128, KT, HD], BF16, tag="wv")
        wo_sb = wpool.tile([128, 4, D], BF16, tag="wo")

        mask_sb = cpool.tile([128, 4, 512], BF16, tag="mask")
        ident_sb = cpool.tile([128, 128], F32R, tag="ident")
        ones_sb = cpool.tile([128, 128], F32R, tag="ones")

        # per-chunk tiles: one [128,512] each per chunk so old-chunk reads
        # never depend on the current chunk's writes (false-dep avoidance)
        krope_cs = [big.tile([128, 512], F32R, tag=f"krope{i}",
                             name=f"krope{i}") for i in range(NCHUNK)]
        vnat_cs = [big.tile([128, 512], F32R, tag=f"vnat{i}",
                            name=f"vnat{i}") for i in range(NCHUNK)]
        ctx_sb = big.tile([128, QH, S], BF16, tag="ctx")   # normalized context^T

        # --- weight group loads (scalar queue; chunk-0 only) ---
        def wq_load(g):  # 4 groups of 8 t-tiles, 1 MB each
            nc.scalar.dma_start(out=wq_sb[:, 8 * g:8 * g + 8, :],
                                in_=wqb_d[:, 8 * g:8 * g + 8, :])

        def wkv_load(g):  # 2 groups of 16 t-tiles each
            nc.scalar.dma_start(out=wk_sb[:, 16 * g:16 * g + 16, :],
                                in_=wkb_d[:, 16 * g:16 * g + 16, :])
            nc.scalar.dma_start(out=wv_sb[:, 16 * g:16 * g + 16, :],
                                in_=wvb_d[:, 16 * g:16 * g + 16, :])

        wq_load(0)
        wkv_load(0)
        wq_load(1)

        def _late_loads(t):
            # chunk-0 only: stream remaining weights + tables
            if t == 2:
                wkv_load(1)
            elif t == 4:
                wq_load(2)
            elif t == 8:
                wq_load(3)
            elif t == 12:
                nc.gpsimd.dma_start(out=mask_sb[:], in_=mask_d[:, :, :])
            elif t == 16:
                nc.gpsimd.dma_start(out=ident_sb[:], in_=ident_d[:, :])
                nc.gpsimd.dma_start(out=ones_sb[:], in_=ones_d[:, :])

        hst_prefetched = {}

        def hst_pair_load(icnk, tp):
            key = (icnk, tp)
            if key in hst_prefetched:
                return hst_prefetched.pop(key)
            h = hsp.tile([128, 2, 512], BF16, tag="hst", name=f"hst{icnk}_{tp}")
            r0 = (icnk * 16 + tp) * 128
            nc.sync.dma_start(out=h[:], in_=hst_d[r0:r0 + 128, :])
            return h

        # ---- fused per-chunk pipeline ----
        for icnk in range(NCHUNK):
            c0, c1 = icnk * 512, (icnk + 1) * 512

            # per-chunk trig slices (gpsimd queue, contiguous per partition)
            cos_t = trig.tile([128, 512], F32, tag="cos", name=f"cos{icnk}")
            sinn_t = trig.tile([128, 512], F32, tag="sinn", name=f"sinn{icnk}")
            nc.gpsimd.dma_start(out=cos_t[:], in_=cos_d[:, icnk, :])
            nc.gpsimd.dma_start(out=sinn_t[:], in_=sinn_d[:, icnk, :])

            # joint qkv projection for this chunk: 6 accumulators (q0..q3, k, v)
            accs = [psum.tile([128, 512], F32, tag="ps", name=f"acc{icnk}_{i}")
                    for i in range(6)]
            for tp in range(16):
                hst_t = hst_pair_load(icnk, tp)
                if icnk == 0:
                    _late_loads(2 * tp)
                for tt in range(2):
                    t = 2 * tp + tt
                    # last iteration: finish k, then q0, then v, so their
                    # evac/rope chains start while the PE drains the rest
                    morder = ((4, 0, 5, 1, 2, 3) if t == KT - 1
                              else (0, 1, 2, 3, 4, 5))
                    for m in morder:
                        if m < 4:
                            lhsT = wq_sb[:, t, m * HD:(m + 1) * HD]
                        elif m == 4:
                            lhsT = wk_sb[:, t, :]
                        else:
                            lhsT = wv_sb[:, t, :]
                        nc.tensor.matmul(accs[m][:], lhsT, hst_t[:, tt, :],
                                         start=(t == 0), stop=(t == KT - 1))

            def evac(m, eng="v"):
                ch = chp.tile([128, 512], F32R, tag="ch", name=f"ch{icnk}_{m}")
                if eng == "s":  # scalar engine: parallel to DVE at boundaries
                    nc.scalar.copy(ch[:], accs[m][:])
                else:
                    nc.vector.tensor_copy(ch[:], accs[m][:])
                return ch

            def rope_into(ch, dest_ap, name):
                # rotate_half via partition-shifted copies; sign folded in sinn
                rot = rotp.tile([128, 512], F32, tag="rot", name=f"rot{name}")
                nc.vector.tensor_copy(rot[0:64, :], ch[64:128, :].bitcast(F32))
                nc.vector.tensor_copy(rot[64:128, :], ch[0:64, :].bitcast(F32))
                t1 = tmpp.tile([128, 512], F32, tag="t1", name=f"t1{name}")
                nc.vector.tensor_mul(t1[:], ch[:].bitcast(F32), cos_t[:])
                t2 = tmpp.tile([128, 512], F32, tag="t2", name=f"t2{name}")
                nc.vector.tensor_mul(t2[:], rot[:], sinn_t[:])
                nc.vector.tensor_add(dest_ap, t1[:], t2[:])

            def rope_q(m):
                qr = qrp.tile([128, 512], F32R, tag="qrp", name=f"qr{icnk}_{m}")
                rope_into(chs[m], qr[:], f"q{icnk}_{m}")
                return qr

            # evacuate ALL psum accumulators upfront (frees banks for the
            # attention tiles; lazy evac deadlocks the 8-slot psum rotation),
            # but compose ropes lazily per head so DVE runs just ahead of PE.
            # evacs for k/q0/v on the scalar engine (idle at boundaries) so
            # the DVE can start rope immediately; q1-3 evac on DVE after
            qrs = [None] * QH
            chs = {}
            if icnk == 0:
                chs[4] = evac(4, "s")
                chs[0] = evac(0, "s")
                chs[5] = evac(5, "s")
                rope_into(chs[4], krope_cs[icnk][:], f"k{icnk}")
                qrs[0] = qrp.tile([128, 512], F32R, tag="qrp",
                                  name=f"qr{icnk}_0")
                rope_into(chs[0], qrs[0][:], f"q{icnk}_0")
            else:
                chs[0] = evac(0, "s")
                chs[4] = evac(4, "s")
                chs[5] = evac(5, "s")
                qrs[0] = qrp.tile([128, 512], F32R, tag="qrp",
                                  name=f"qr{icnk}_0")
                rope_into(chs[0], qrs[0][:], f"q{icnk}_0")
                rope_into(chs[4], krope_cs[icnk][:], f"k{icnk}")
            for m in (1, 2, 3):
                chs[m] = evac(m)

            # v transpose: 4 sub-blocks into one psum bank, then DVE to vnat
            ch_v = chs[5]
            vt_ps = psum.tile([128, 512], F32R, tag="ps", name=f"vt{icnk}")
            for tt in range(4):
                nc.tensor.matmul(vt_ps[:, tt * 128:(tt + 1) * 128],
                                 ch_v[:, tt * 128:(tt + 1) * 128],
                                 ident_sb[:], is_transpose=True,
                                 start=True, stop=True)
            for tt in range(4):
                nc.vector.tensor_copy(vnat_cs[icnk][:, tt * 128:(tt + 1) * 128],
                                      vt_ps[:, tt * 128:(tt + 1) * 128])

            # attention for the 4 heads, query chunk = icnk (keys 0..4icnk+3)
            def attention(h):
                qr = qrs[h]
                ctx_acc = psum.tile([128, 512], F32, tag="ps",
                                    name=f"ctx{icnk}_{h}")
                acc_rs = rsp.tile([128, 512], F32R, tag="rs",
                                  name=f"rs{icnk}_{h}")
                jt_max = icnk * 4 + 3
                pending = []
                LOOKAHEAD = 2

                def consume(item, last):
                    jt, lo, pT = item
                    nc.tensor.matmul(
                        ctx_acc[:, lo:512],
                        vnat_cs[jt // 4][:, (jt % 4) * 128:(jt % 4 + 1) * 128],
                        pT[:, lo:512],
                        start=(jt == 0), stop=last,
                        skip_group_check=True)

                for jt in range(jt_max + 1):
                    r = jt - icnk * 4
                    lo = 0 if r <= 0 else (128 if r == 1 else 256)
                    sT = psum.tile([128, 512], F32, tag="ps",
                                   name=f"sT{icnk}_{h}_{jt}")
                    nc.tensor.matmul(
                        sT[:, lo:512],
                        krope_cs[jt // 4][:, (jt % 4) * 128:(jt % 4 + 1) * 128],
                        qr[:, lo:512], start=True, stop=True)
                    if len(pending) >= LOOKAHEAD:
                        consume(pending.pop(0), False)
                    if r >= 0:  # diagonal: apply causal mask
                        nc.vector.tensor_add(sT[:, lo:512], sT[:, lo:512],
                                             mask_sb[:, r, lo:512])
                    pT = ptp.tile([128, 512], F32R, tag="pt",
                                  name=f"pt{icnk}_{h}_{jt}")
                    nc.scalar.activation(out=pT[:, lo:512], in_=sT[:, lo:512],
                                         func=mybir.ActivationFunctionType.Exp,
                                         scale=INV_SQRT_D)
                    # rowsum accumulate on DVE
                    if jt == 0:
                        nc.vector.tensor_copy(acc_rs[:, :], pT[:, :])
                    else:
                        nc.vector.tensor_add(acc_rs[:, lo:512],
                                             acc_rs[:, lo:512], pT[:, lo:512])
                    pending.append((jt, lo, pT))
                while pending:
                    consume(pending.pop(0), len(pending) == 0)

                # rowsum + normalize; broadcast on gpsimd, off the PE path
                rs_ps = psum.tile([1, 512], F32, tag="ps",
                                  name=f"rsps{icnk}_{h}")
                nc.tensor.matmul(rs_ps[:], ones_sb[:, 0:1], acc_rs[:],
                                 start=True, stop=True)
                recip = smal.tile([1, 512], F32, tag="recip")
                nc.vector.reciprocal_approx_fast(recip[:], rs_ps[:])
                rb = rbp.tile([128, 512], F32, tag="rb", name=f"rb{icnk}_{h}")
                nc.gpsimd.partition_broadcast(rb[:], recip[:])
                nc.vector.tensor_mul(ctx_sb[:, h, c0:c1], ctx_acc[:], rb[:])

            for h in range(QH):
                if h + 1 < QH:
                    qrs[h + 1] = rope_q(h + 1)  # DVE runs ahead of the PE here
                attention(h)
                if icnk < NCHUNK - 1 and h == 1:
                    # prefetch next chunk's first hst pairs during attention
                    for tpre in range(3):
                        hh = hsp.tile([128, 2, 512], BF16, tag="hst",
                                      name=f"hst{icnk + 1}_{tpre}")
                        r0 = ((icnk + 1) * 16 + tpre) * 128
                        nc.sync.dma_start(out=hh[:], in_=hst_d[r0:r0 + 128, :])
                        hst_prefetched[(icnk + 1, tpre)] = hh
                if icnk == NCHUNK - 2 and h == 2:
                    # load bf16 Wo (own slot) well before the o-projection
                    for jt in range(4):
                        nc.gpsimd.dma_start(
                            out=wo_sb[:, jt, :],
                            in_=wob_d[jt * 128:(jt + 1) * 128, :])

        # ---- partial o-projection from SBUF ctx (bf16), bf16 out ----
        # out written in ec-PAIRS on alternating queues: the per-DMA issue
        # cost (~1.4us) would otherwise gate the o-proj phase
        out_r = out_d[:, :].rearrange("(n p) f -> p n f", p=128)
        for st in range(NJT):
            for ecp in range(4):  # pairs of ec
                ot = osb.tile([128, 2, 512], BF16, tag="ot",
                              name=f"ot{st}_{ecp}")
                for half in range(2):
                    ec = ecp * 2 + half
                    oacc = psum.tile([128, 512], F32, tag="ps",
                                     name=f"o{st}_{ec}")
                    for jt in range(4):
                        nc.tensor.matmul(
                            oacc[:],
                            ctx_sb[:, jt, st * 128:(st + 1) * 128],
                            wo_sb[:, jt, ec * 512:(ec + 1) * 512],
                            start=(jt == 0), stop=(jt == 3))
                    nc.scalar.copy(ot[:, half, :], oacc[:])
                n0 = st * 8 + ecp * 2
                eng = nc.sync if ecp % 2 == 0 else nc.gpsimd
                eng.dma_start(out=out_r[:, n0:n0 + 2, :], in_=ot[:])

    nc.finalize()
    return nc


_NC_CACHE = None


def _host_tables():
    inv_freq = 1.0 / (ROPE_BASE ** (np.arange(0, HD, 2, dtype=np.float32) / HD))
    pos = np.arange(S, dtype=np.float32)
    freqs = pos[:, None] * inv_freq[None, :].astype(np.float32)   # [S, 64]
    emb = np.concatenate([freqs, freqs], axis=1).astype(np.float32)  # [S, 128]
    cosT = np.ascontiguousarray(np.cos(emb).astype(np.float32).T)  # [128, S]
    sinT = np.ascontiguousarray(np.sin(emb).astype(np.float32).T)
    sinN = sinT.copy()
    sinN[0:64, :] *= -1.0  # sign folded: rot_abs[0:64]=x2, [64:]=x1

    # per-chunk partition-major trig: [128, NCHUNK, 512]
    cosPM = np.ascontiguousarray(cosT.reshape(128, NCHUNK, 512))
    sinnPM = np.ascontiguousarray(sinN.reshape(128, NCHUNK, 512))

    # causal mask for diagonal blocks, transposed orientation [jp, r, if]
    jp = np.arange(128)[:, None, None]
    r = np.arange(4)[None, :, None]
    iF = np.arange(512)[None, None, :]
    mask = np.where(r * 128 + jp <= iF, 0.0, NEG_INF).astype(np.float32)
    mask_bf = mask.astype(ml_dtypes.bfloat16)

    ident = np.eye(128, dtype=np.float32)
    ones = np.ones((128, 128), dtype=np.float32)
    return cosPM, sinnPM, mask_bf, ident, ones


def kernel(hidden_states, Wq, Wk, Wv, Wo):
    global _NC_CACHE
    if _NC_CACHE is None:
        _NC_CACHE = _build_nc()
    nc = _NC_CACHE

    hs = np.asarray(hidden_states, dtype=np.float32)
    B = hs.shape[0]
    assert hs.shape == (B, S, D)
    hst = np.ascontiguousarray(hs[0].T).astype(ml_dtypes.bfloat16)  # [D, S]
    # pair-tiled: row (icnk*16 + tp)*128 + p = [t=2tp | t=2tp+1] halves
    hst_t = np.ascontiguousarray(
        hst.reshape(16, 2, 128, NCHUNK, 512).transpose(3, 0, 2, 1, 4)
    ).reshape(NCHUNK * 16 * 128, 1024)
    cosPM, sinnPM, mask_bf, ident, ones = _host_tables()

    Wq = np.asarray(Wq, dtype=np.float32)
    Wk = np.asarray(Wk, dtype=np.float32)
    Wv = np.asarray(Wv, dtype=np.float32)
    Wo = np.asarray(Wo, dtype=np.float32)

    def pm_bf16(w):  # [D, m] -> partition-major [128, KT, m] bf16
        m = w.shape[1]
        return np.ascontiguousarray(
            w.reshape(KT, 128, m).transpose(1, 0, 2)).astype(ml_dtypes.bfloat16)

    in_maps = []
    for c in range(NCORES):
        in_maps.append({
            "hst": hst_t,
            "wqb": pm_bf16(Wq[:, c * QH * HD:(c + 1) * QH * HD]),
            "wkb": pm_bf16(Wk[:, c * HD:(c + 1) * HD]),
            "wvb": pm_bf16(Wv[:, c * HD:(c + 1) * HD]),
            "wob": np.ascontiguousarray(
                Wo[c * QH * HD:(c + 1) * QH * HD, :]).astype(ml_dtypes.bfloat16),
            "cost": cosPM,
            "sinn": sinnPM,
            "maskt": mask_bf,
            "ident": ident,
            "ones": ones,
        })

    import os
    trace = os.environ.get("KERNEL_TRACE") == "1"
    if trace:
        try:
            import antenv.axon_hooks  # noqa: F401  (profiling hook, optional)
        except ImportError:
            trace = False
    res = run_bass_kernel_spmd(nc, in_maps, list(range(NCORES)), trace=trace)
    if trace:
        kernel.last_results = res

    acc = np.zeros((NJT, 8, 128, 512), dtype=np.float64)
    for c in range(NCORES):
        acc += res.results[c]["out"].astype(np.float64).reshape(NJT, 8, 128, 512)
    # [st, ec, p, f] -> [st, p, ec, f] -> [S, D]
    out = acc.transpose(0, 2, 1, 3).reshape(S, D).astype(np.float32)
    return out.reshape(B, S, D)


# revision 19
# speedup vs baseline: 1.2160x; 1.1583x over previous
"""GQA attention kernel (B=1, S=2048, D=4096, 32 Q heads / 8 KV heads, RoPE,
causal) for 8 Trainium2 NeuronCores.

Sharding: tensor-parallel over heads. Core c owns Q heads 4c..4c+3 and KV head
c (whole GQA group), computes its context slice and a partial o-projection
(rows 512c..512c+511 of Wo); the host sums the 8 partial outputs.

v2 changes vs baseline:
- all DRAM tensors pre-tiled on host so every DMA is per-partition contiguous
- QKV weights shipped bf16, upconverted to fp32r on DVE (halves weight DMA)
- rowsum via DVE accumulation of exp tiles + one ones-column matmul per
  (head, chunk) instead of a 512-cycle matmul per key tile
- rotate_half via DVE partition-shifted copies + sign-folded sin table
  (no PE matmul)
- causal diagonal tiles width-restricted (N>=256 kept for fp32r full rate)
- engine split: PE matmul only / scalar exp only / DVE elementwise
- o-projection from SBUF-resident bf16 context, bf16 Wo, bf16 output
  (host upconverts + sums); no ctx DRAM roundtrip
"""
import numpy as np
import ml_dtypes
from contextlib import ExitStack

try:  # reuse compiled executables across processes when possible
    import jax
    jax.config.update("jax_compilation_cache_dir", "/tmp/jax_comp_cache")
    jax.config.update("jax_persistent_cache_min_entry_size_bytes", -1)
    jax.config.update("jax_persistent_cache_min_compile_time_secs", 1.0)
except Exception:
    pass

import concourse.bacc as bacc
import concourse.tile as tile
import concourse.mybir as mybir
from concourse.bass_utils import run_bass_kernel_spmd

F32 = mybir.dt.float32
F32R = mybir.dt.float32r
BF16 = mybir.dt.bfloat16

S = 2048            # sequence length
D = 4096            # hidden dim
HD = 128            # head dim
NCORES = 8
QH = 4              # q heads per core
KT = D // 128       # 32 contraction tiles for the projections
NCHUNK = S // 512   # 4 sequence chunks of 512
NJT = S // 128      # 16 seq tiles of 128
INV_SQRT_D = float(1.0 / np.sqrt(np.float32(HD)))
NEG_INF = -3.4e38
ROPE_BASE = 10000.0


def round_fp32r(x: np.ndarray) -> np.ndarray:
    """Round fp32 to fp32r (11 mantissa bits, RNE); low 12 bits zeroed."""
    u = np.ascontiguousarray(x, dtype=np.float32).view(np.uint32)
    keep = (u >> 12) & np.uint32(1)
    u = u + np.uint32(0x7FF) + keep
    u = u & np.uint32(0xFFFFF000)
    return u.view(np.float32)


def _build_nc():
    nc = bacc.Bacc(None)

    # pre-tiled bf16 inputs: hst rows ((icnk*16 + tp)*128 + p) hold the
    # t-PAIR (2*tp, 2*tp+1) side by side -> one DMA per two tiles
    hst_d = nc.dram_tensor("hst", [NCHUNK * 16 * 128, 1024], BF16, kind="ExternalInput")
    # weights bf16, partition-major [128, KT, m] so group loads are contiguous
    wqb_d = nc.dram_tensor("wqb", [128, KT, QH * HD], BF16, kind="ExternalInput")
    wkb_d = nc.dram_tensor("wkb", [128, KT, HD], BF16, kind="ExternalInput")
    wvb_d = nc.dram_tensor("wvb", [128, KT, HD], BF16, kind="ExternalInput")
    wob_d = nc.dram_tensor("wob", [4 * 128, D], BF16, kind="ExternalInput")
    # cos / sign-folded sin, partition-major per chunk [128, NCHUNK, 512]
    cos_d = nc.dram_tensor("cost", [128, NCHUNK, 512], F32, kind="ExternalInput")
    sinn_d = nc.dram_tensor("sinn", [128, NCHUNK, 512], F32, kind="ExternalInput")
    mask_d = nc.dram_tensor("maskt", [128, 4, 512], BF16, kind="ExternalInput")
    ident_d = nc.dram_tensor("ident", [128, 128], F32R, kind="ExternalInput")
    ones_d = nc.dram_tensor("ones", [128, 128], F32R, kind="ExternalInput")
    # output bf16, tiled rows ((st*8 + ec)*128 + p)
    out_d = nc.dram_tensor("out", [NJT * 8 * 128, 512], BF16, kind="ExternalOutput")

    with tile.TileContext(nc) as tc, ExitStack() as ctx:
        wpool = ctx.enter_context(tc.tile_pool(name="wpool", bufs=1))
        cpool = ctx.enter_context(tc.tile_pool(name="cpool", bufs=1))
        big = ctx.enter_context(tc.tile_pool(name="bigacts", bufs=1))
        trig = ctx.enter_context(tc.tile_pool(name="trig", bufs=2))
        hsp = ctx.enter_context(tc.tile_pool(name="hsp", bufs=4))
        chp = ctx.enter_context(tc.tile_pool(name="chp", bufs=4))
        rotp = ctx.enter_context(tc.tile_pool(name="rotp", bufs=1))
        qrp = ctx.enter_context(tc.tile_pool(name="qrp", bufs=4))
        tmpp = ctx.enter_context(tc.tile_pool(name="tmpp", bufs=1))
        ptp = ctx.enter_context(tc.tile_pool(name="ptp", bufs=5))
        rsp = ctx.enter_context(tc.tile_pool(name="rsp", bufs=2))
        smal = ctx.enter_context(tc.tile_pool(name="smal", bufs=1))
        rbp = ctx.enter_context(tc.tile_pool(name="rbp", bufs=2))
        osb = ctx.enter_context(tc.tile_pool(name="osb", bufs=4))
        psum = ctx.enter_context(tc.tile_pool(name="psum", bufs=8, space="PSUM"))

        # ---- resident weights & constants (bf16, loaded directly) ----
        wq_sb = wpool.tile([128, KT, QH * HD], BF16, tag="wq")
        wk_sb = wpool.tile([128, KT, HD], BF16, tag="wk")
        wv_sb = wpool.tile([128, KT, HD], BF16, tag="wv")
        wo_sb = wpool.tile([128, 4, D], BF16, tag="wo")

        mask_sb = cpool.tile([128, 4, 512], BF16, tag="mask")
        ident_sb = cpool.tile([128, 128], F32R, tag="ident")
        ones_sb = cpool.tile([128, 128], F32R, tag="ones")

        # per-chunk tiles: one [128,512] each per chunk so old-chunk reads
        # never depend on the current chunk's writes (false-dep avoidance)
        krope_cs = [big.tile([128, 512], F32R, tag=f"krope{i}",
                             name=f"krope{i}") for i in range(NCHUNK)]
        vnat_cs = [big.tile([128, 512], F32R, tag=f"vnat{i}",
                            name=f"vnat{i}") for i in range(NCHUNK)]
        ctx_cs = [big.tile([128, QH, 512], BF16, tag=f"ctx{i}",
                           name=f"ctx{i}") for i in range(NCHUNK)]

        # --- weight group loads (scalar queue; chunk-0 only) ---
        def wq_load(g):  # 4 groups of 8 t-tiles, 1 MB each
            nc.scalar.dma_start(out=wq_sb[:, 8 * g:8 * g + 8, :],
                                in_=wqb_d[:, 8 * g:8 * g + 8, :])

        def wkv_load(g):  # 2 groups of 16 t-tiles each
            nc.scalar.dma_start(out=wk_sb[:, 16 * g:16 * g + 16, :],
                                in_=wkb_d[:, 16 * g:16 * g + 16, :])
            nc.scalar.dma_start(out=wv_sb[:, 16 * g:16 * g + 16, :],
                                in_=wvb_d[:, 16 * g:16 * g + 16, :])

        wq_load(0)
        wkv_load(0)
        wq_load(1)

        def _late_loads(t):
            # chunk-0 only: stream remaining weights + tables
            if t == 2:
                wkv_load(1)
            elif t == 4:
                wq_load(2)
            elif t == 8:
                wq_load(3)
            elif t == 12:
                nc.gpsimd.dma_start(out=mask_sb[:], in_=mask_d[:, :, :])
            elif t == 16:
                nc.gpsimd.dma_start(out=ident_sb[:], in_=ident_d[:, :])
                nc.gpsimd.dma_start(out=ones_sb[:], in_=ones_d[:, :])

        hst_prefetched = {}

        # o-projection unit emitter: (st, ec-pair) -> 8 matmuls + 2 copies +
        # one paired DMA. Used as PE filler during attention and for the
        # final drain.
        out_r = out_d[:, :].rearrange("(n p) f -> p n f", p=128)
        _ounits = [(st, ecp) for st in range(NJT) for ecp in range(4)]
        _ostate = {"i": 0, "q": 0}

        def emit_oproj(budget, max_st, filler):
            n = 0
            while n < budget and _ostate["i"] < len(_ounits):
                st, ecp = _ounits[_ostate["i"]]
                if st > max_st:
                    break
                _ostate["i"] += 1
                ot = osb.tile([128, 2, 512], BF16, tag="ot",
                              name=f"ot{st}_{ecp}")
                for half in range(2):
                    ec = ecp * 2 + half
                    oacc = psum.tile([128, 512], F32, tag="ps",
                                     name=f"o{st}_{ec}")
                    for jt in range(4):
                        nc.tensor.matmul(
                            oacc[:],
                            ctx_cs[st // 4][:, jt,
                                            (st % 4) * 128:(st % 4 + 1) * 128],
                            wo_sb[:, jt, ec * 512:(ec + 1) * 512],
                            start=(jt == 0), stop=(jt == 3))
                    if filler and half == 0:
                        nc.vector.tensor_copy(ot[:, half, :], oacc[:])
                    else:
                        nc.scalar.copy(ot[:, half, :], oacc[:])
                n0 = st * 8 + ecp * 2
                eng = nc.sync if _ostate["q"] % 2 == 0 else nc.gpsimd
                _ostate["q"] += 1
                eng.dma_start(out=out_r[:, n0:n0 + 2, :], in_=ot[:])
                n += 1
            return n

        def hst_pair_load(icnk, tp):
            key = (icnk, tp)
            if key in hst_prefetched:
                return hst_prefetched.pop(key)
            h = hsp.tile([128, 2, 512], BF16, tag="hst", name=f"hst{icnk}_{tp}")
            r0 = (icnk * 16 + tp) * 128
            nc.sync.dma_start(out=h[:], in_=hst_d[r0:r0 + 128, :])
            return h

        # ---- fused per-chunk pipeline ----
        for icnk in range(NCHUNK):
            c0, c1 = icnk * 512, (icnk + 1) * 512

            # per-chunk trig slices (gpsimd queue, contiguous per partition)
            cos_t = trig.tile([128, 512], F32, tag="cos", name=f"cos{icnk}")
            sinn_t = trig.tile([128, 512], F32, tag="sinn", name=f"sinn{icnk}")
            nc.gpsimd.dma_start(out=cos_t[:], in_=cos_d[:, icnk, :])
            nc.gpsimd.dma_start(out=sinn_t[:], in_=sinn_d[:, icnk, :])

            # joint qkv projection for this chunk: 6 accumulators (q0..q3, k, v)
            accs = [psum.tile([128, 512], F32, tag="ps", name=f"acc{icnk}_{i}")
                    for i in range(6)]
            for tp in range(16):
                hst_t = hst_pair_load(icnk, tp)
                if icnk == 0:
                    _late_loads(2 * tp)
                elif icnk == 1 and tp in (4, 8):
                    jt0 = 0 if tp == 4 else 2
                    for jt in (jt0, jt0 + 1):
                        nc.gpsimd.dma_start(
                            out=wo_sb[:, jt, :],
                            in_=wob_d[jt * 128:(jt + 1) * 128, :])
                for tt in range(2):
                    t = 2 * tp + tt
                    # last iteration: finish k, then q0, then v, so their
                    # evac/rope chains start while the PE drains the rest
                    morder = ((4, 0, 5, 1, 2, 3) if t == KT - 1
                              else (0, 1, 2, 3, 4, 5))
                    for m in morder:
                        if m < 4:
                            lhsT = wq_sb[:, t, m * HD:(m + 1) * HD]
                        elif m == 4:
                            lhsT = wk_sb[:, t, :]
                        else:
                            lhsT = wv_sb[:, t, :]
                        nc.tensor.matmul(accs[m][:], lhsT, hst_t[:, tt, :],
                                         start=(t == 0), stop=(t == KT - 1))

            def evac(m, eng="v"):
                ch = chp.tile([128, 512], F32R, tag="ch", name=f"ch{icnk}_{m}")
                if eng == "s":  # scalar engine: parallel to DVE at boundaries
                    nc.scalar.copy(ch[:], accs[m][:])
                else:
                    nc.vector.tensor_copy(ch[:], accs[m][:])
                return ch

            def rope_into(ch, dest_ap, name):
                # rotate_half via partition-shifted copies; sign folded in sinn
                rot = rotp.tile([128, 512], F32, tag="rot", name=f"rot{name}")
                nc.vector.tensor_copy(rot[0:64, :], ch[64:128, :].bitcast(F32))
                nc.vector.tensor_copy(rot[64:128, :], ch[0:64, :].bitcast(F32))
                t1 = tmpp.tile([128, 512], F32, tag="t1", name=f"t1{name}")
                nc.vector.tensor_mul(t1[:], ch[:].bitcast(F32), cos_t[:])
                t2 = tmpp.tile([128, 512], F32, tag="t2", name=f"t2{name}")
                nc.vector.tensor_mul(t2[:], rot[:], sinn_t[:])
                nc.vector.tensor_add(dest_ap, t1[:], t2[:])

            def rope_q(m):
                qr = qrp.tile([128, 512], F32R, tag="qrp", name=f"qr{icnk}_{m}")
                rope_into(chs[m], qr[:], f"q{icnk}_{m}")
                return qr

            # evacuate ALL psum accumulators upfront (frees banks for the
            # attention tiles; lazy evac deadlocks the 8-slot psum rotation),
            # but compose ropes lazily per head so DVE runs just ahead of PE.
            # evacs for k/q0/v on the scalar engine (idle at boundaries) so
            # the DVE can start rope immediately; q1-3 evac on DVE after
            qrs = [None] * QH
            chs = {}
            if icnk == 0:
                chs[4] = evac(4, "s")
                chs[0] = evac(0, "s")
                chs[5] = evac(5, "s")
                rope_into(chs[4], krope_cs[icnk][:], f"k{icnk}")
                qrs[0] = qrp.tile([128, 512], F32R, tag="qrp",
                                  name=f"qr{icnk}_0")
                rope_into(chs[0], qrs[0][:], f"q{icnk}_0")
            else:
                chs[0] = evac(0, "s")
                chs[4] = evac(4, "s")
                chs[5] = evac(5, "s")
                qrs[0] = qrp.tile([128, 512], F32R, tag="qrp",
                                  name=f"qr{icnk}_0")
                rope_into(chs[0], qrs[0][:], f"q{icnk}_0")
                rope_into(chs[4], krope_cs[icnk][:], f"k{icnk}")
            ch_v = chs[5]
            vt_ps = psum.tile([128, 512], F32R, tag="ps", name=f"vt{icnk}")
            for tt in range(4):
                nc.tensor.matmul(vt_ps[:, tt * 128:(tt + 1) * 128],
                                 ch_v[:, tt * 128:(tt + 1) * 128],
                                 ident_sb[:], is_transpose=True,
                                 start=True, stop=True)
            for tt in range(4):
                nc.vector.tensor_copy(vnat_cs[icnk][:, tt * 128:(tt + 1) * 128],
                                      vt_ps[:, tt * 128:(tt + 1) * 128])
            for m in (1, 2, 3):
                chs[m] = evac(m)
            if icnk >= 1:
                # fill the rope-chain wait with an o-proj unit
                emit_oproj(1, 4 * icnk - 1, filler=True)

            # attention for the 4 heads, query chunk = icnk (keys 0..4icnk+3)
            def attention(h):
                qr = qrs[h]
                ctx_acc = psum.tile([128, 512], F32, tag="ps",
                                    name=f"ctx{icnk}_{h}")
                acc_rs = rsp.tile([128, 512], F32R, tag="rs",
                                  name=f"rs{icnk}_{h}")
                jt_max = icnk * 4 + 3
                pending = []
                LOOKAHEAD = 2

                def consume(item, last):
                    jt, lo, pT = item
                    nc.tensor.matmul(
                        ctx_acc[:, lo:512],
                        vnat_cs[jt // 4][:, (jt % 4) * 128:(jt % 4 + 1) * 128],
                        pT[:, lo:512],
                        start=(jt == 0), stop=last,
                        skip_group_check=True)

                for jt in range(jt_max + 1):
                    r = jt - icnk * 4
                    lo = 0 if r <= 0 else (128 if r == 1 else 256)
                    sT = psum.tile([128, 512], F32, tag="ps",
                                   name=f"sT{icnk}_{h}_{jt}")
                    nc.tensor.matmul(
                        sT[:, lo:512],
                        krope_cs[jt // 4][:, (jt % 4) * 128:(jt % 4 + 1) * 128],
                        qr[:, lo:512], start=True, stop=True)
                    if len(pending) >= LOOKAHEAD:
                        consume(pending.pop(0), False)
                    if r >= 0:  # diagonal: apply causal mask
                        nc.vector.tensor_add(sT[:, lo:512], sT[:, lo:512],
                                             mask_sb[:, r, lo:512])
                    pT = ptp.tile([128, 512], F32R, tag="pt",
                                  name=f"pt{icnk}_{h}_{jt}")
                    nc.scalar.activation(out=pT[:, lo:512], in_=sT[:, lo:512],
                                         func=mybir.ActivationFunctionType.Exp,
                                         scale=INV_SQRT_D)
                    # rowsum accumulate on DVE
                    if jt == 0:
                        nc.vector.tensor_copy(acc_rs[:, :], pT[:, :])
                    else:
                        nc.vector.tensor_add(acc_rs[:, lo:512],
                                             acc_rs[:, lo:512], pT[:, lo:512])
                    pending.append((jt, lo, pT))
                while pending:
                    consume(pending.pop(0), len(pending) == 0)

                # rowsum + normalize; broadcast on gpsimd, off the PE path
                rs_ps = psum.tile([1, 512], F32, tag="ps",
                                  name=f"rsps{icnk}_{h}")
                nc.tensor.matmul(rs_ps[:], ones_sb[:, 0:1], acc_rs[:],
                                 start=True, stop=True)
                recip = smal.tile([1, 512], F32, tag="recip")
                nc.vector.reciprocal_approx_fast(recip[:], rs_ps[:])
                rb = rbp.tile([128, 512], F32, tag="rb", name=f"rb{icnk}_{h}")
                nc.gpsimd.partition_broadcast(rb[:], recip[:])
                nc.vector.tensor_mul(ctx_cs[icnk][:, h, :], ctx_acc[:], rb[:])

            for h in range(QH):
                if h + 1 < QH:
                    qrs[h + 1] = rope_q(h + 1)  # DVE runs ahead of the PE here
                attention(h)
                if icnk >= 1:
                    emit_oproj(1 if icnk == 1 else 2, 4 * icnk - 1,
                               filler=True)
                if icnk < NCHUNK - 1 and h == 1:
                    # prefetch next chunk's first hst pairs during attention
                    for tpre in range(3):
                        hh = hsp.tile([128, 2, 512], BF16, tag="hst",
                                      name=f"hst{icnk + 1}_{tpre}")
                        r0 = ((icnk + 1) * 16 + tpre) * 128
                        nc.sync.dma_start(out=hh[:], in_=hst_d[r0:r0 + 128, :])
                        hst_prefetched[(icnk + 1, tpre)] = hh

        # ---- drain remaining o-projection units ----
        emit_oproj(len(_ounits), NJT - 1, filler=False)

    nc.finalize()
    return nc


_NC_CACHE = None


def _host_tables():
    inv_freq = 1.0 / (ROPE_BASE ** (np.arange(0, HD, 2, dtype=np.float32) / HD))
    pos = np.arange(S, dtype=np.float32)
    freqs = pos[:, None] * inv_freq[None, :].astype(np.float32)   # [S, 64]
    emb = np.concatenate([freqs, freqs], axis=1).astype(np.float32)  # [S, 128]
    cosT = np.ascontiguousarray(np.cos(emb).astype(np.float32).T)  # [128, S]
    sinT = np.ascontiguousarray(np.sin(emb).astype(np.float32).T)
    sinN = sinT.copy()
    sinN[0:64, :] *= -1.0  # sign folded: rot_abs[0:64]=x2, [64:]=x1

    # per-chunk partition-major trig: [128, NCHUNK, 512]
    cosPM = np.ascontiguousarray(cosT.reshape(128, NCHUNK, 512))
    sinnPM = np.ascontiguousarray(sinN.reshape(128, NCHUNK, 512))

    # causal mask for diagonal blocks, transposed orientation [jp, r, if]
    jp = np.arange(128)[:, None, None]
    r = np.arange(4)[None, :, None]
    iF = np.arange(512)[None, None, :]
    mask = np.where(r * 128 + jp <= iF, 0.0, NEG_INF).astype(np.float32)
    mask_bf = mask.astype(ml_dtypes.bfloat16)

    ident = np.eye(128, dtype=np.float32)
    ones = np.ones((128, 128), dtype=np.float32)
    return cosPM, sinnPM, mask_bf, ident, ones


def kernel(hidden_states, Wq, Wk, Wv, Wo):
    global _NC_CACHE
    if _NC_CACHE is None:
        _NC_CACHE = _build_nc()
    nc = _NC_CACHE

    hs = np.asarray(hidden_states, dtype=np.float32)
    B = hs.shape[0]
    assert hs.shape == (B, S, D)
    hst = np.ascontiguousarray(hs[0].T).astype(ml_dtypes.bfloat16)  # [D, S]
    # pair-tiled: row (icnk*16 + tp)*128 + p = [t=2tp | t=2tp+1] halves
    hst_t = np.ascontiguousarray(
        hst.reshape(16, 2, 128, NCHUNK, 512).transpose(3, 0, 2, 1, 4)
    ).reshape(NCHUNK * 16 * 128, 1024)
    cosPM, sinnPM, mask_bf, ident, ones = _host_tables()

    Wq = np.asarray(Wq, dtype=np.float32)
    Wk = np.asarray(Wk, dtype=np.float32)
    Wv = np.asarray(Wv, dtype=np.float32)
    Wo = np.asarray(Wo, dtype=np.float32)

    def pm_bf16(w):  # [D, m] -> partition-major [128, KT, m] bf16
        m = w.shape[1]
        return np.ascontiguousarray(
            w.reshape(KT, 128, m).transpose(1, 0, 2)).astype(ml_dtypes.bfloat16)

    in_maps = []
    for c in range(NCORES):
        in_maps.append({
            "hst": hst_t,
            "wqb": pm_bf16(Wq[:, c * QH * HD:(c + 1) * QH * HD]),
            "wkb": pm_bf16(Wk[:, c * HD:(c + 1) * HD]),
            "wvb": pm_bf16(Wv[:, c * HD:(c + 1) * HD]),
            "wob": np.ascontiguousarray(
                Wo[c * QH * HD:(c + 1) * QH * HD, :]).astype(ml_dtypes.bfloat16),
            "cost": cosPM,
            "sinn": sinnPM,
            "maskt": mask_bf,
            "ident": ident,
            "ones": ones,
        })

    import os
    trace = os.environ.get("KERNEL_TRACE") == "1"
    if trace:
        try:
            import antenv.axon_hooks  # noqa: F401  (profiling hook, optional)
        except ImportError:
            trace = False
    res = run_bass_kernel_spmd(nc, in_maps, list(range(NCORES)), trace=trace)
    if trace:
        kernel.last_results = res

    acc = np.zeros((NJT, 8, 128, 512), dtype=np.float64)
    for c in range(NCORES):
        acc += res.results[c]["out"].astype(np.float64).reshape(NJT, 8, 128, 512)
    # [st, ec, p, f] -> [st, p, ec, f] -> [S, D]
    out = acc.transpose(0, 2, 1, 3).reshape(S, D).astype(np.float32)
    return out.reshape(B, S, D)
